# revision 32
# baseline (speedup 1.0000x reference)
"""Trainium2 Bass kernel for nn_ContrastiveLoss (SCAN t2i contrastive loss).

Strategy (caption-sharded across 8 cores, per the sharding hint):
  - Each core holds all B=128 images and 16 captions; per (image, caption)
    pair the Gram-matrix identity avoids materialising the weighted context:
        P1[w] = sum_r E*G,  P2[w] = e_w^T Mi e_w,  Mi = im_i @ im_i^T
    with E = exp(9*An), An = lrelu(G)/||lrelu(G)||_words.  The softmax
    denominator S cancels in row_sim, so  rs = P1 * w1inv * rsqrt(P2).
  - Ragged caption packing: each caption's valid words (padded to a multiple
    of 4) are packed contiguously; LPT assignment balances the 8 cores so all
    packed words fit in WCAP=448 columns.  Per-caption segment reductions are
    done as [DMA-xbar transpose -> indicator matmul].
  - v2 restructure (vs the 270us baseline):
      * LeakyReLU is fused into the PSUM evacuation on ACT (alpha=0.1);
        raw G is evacuated bf16 by a second ACT op.  The DVE never runs the
        lrelu STT and GPSIMD only squares.
      * the whole post-G elementwise chain is bf16 so DVE tensor_tensor ops
        run in 2x_1P mode; ups/rcp_row matmuls write bf16 PSUM.
      * the [108,112] unit-sum transpose moved from the PE to the DMA xbar
        (dma_start_transpose), killing the PE wait on the DVE reduce chain.
      * ln/exp rsqrt small ops are batched over chunks of 2 triples.
      * 8-stage software pipeline gives every PE instruction >=1 full step
        of slack so the HAM clock gate stays at 8/8 (the baseline's
        recurring 600-900ns PE stalls held it at 4/8 for 82% of the run).
  - sqrt is computed as exp(-0.5*ln(x)) so the ACT engine stays on the single
    `natural_log_exp_and_others` table (which also holds leaky_relu + copy).
  - Kernel returns raw per-(slot,image) LSE sums; host applies log/6 and the
    tiny (B,B) hinge loss.
"""

import json

import numpy as np
import ml_dtypes

import concourse.bass as bass
import concourse.mybir as mybir
import concourse.tile as tile
from concourse.bass_utils import run_bass_kernel_spmd


def _split_waits(bir_bytes, maxw=1):
    """Walrus in this toolchain accepts only `maxw` sync-waits per
    instruction; hoist extras onto preceding 1-wait Drain no-ops."""
    bir = json.loads(bir_bytes)
    for fn in bir["functions"]:
        for blk in fn["blocks"]:
            out = []
            for inst in blk["instructions"]:
                si = inst.get("sync_info") or {}
                ow = si.get("on_wait") or []
                if len(ow) > maxw:
                    head, tail = ow[:-maxw], ow[-maxw:]
                    for j, w in enumerate(head):
                        out.append({"debug": inst.get("debug"),
                                    "engine": inst["engine"], "ins": [],
                                    "is_reset_sema": False,
                                    "name": f"{inst['name']}-w{j}",
                                    "opcode": "NoOp", "outs": [],
                                    "sync_info": {"on_update": [],
                                                  "on_wait": [w]}})
                    si["on_wait"] = tail
                out.append(inst)
            blk["instructions"] = out
    return json.dumps(bir).encode()


F32 = mybir.dt.float32
BF16 = mybir.dt.bfloat16
AF = mybir.ActivationFunctionType
ALU = mybir.AluOpType

LAMBDA_SOFTMAX = 9.0
LAMBDA_LSE = 6.0
MARGIN = 0.2

B, R, W, D = 128, 36, 50, 1024
NCORES = 8
NSLOT = B // NCORES         # caption slots per core
IMG_PAD = 129               # 43 triples of 3 images
NT = IMG_PAD // 3           # 43
TRIP = 3
PT = TRIP * R               # 108 partitions per triple
KD = D // 128               # 8 contraction chunks
GR = 4                      # word-padding granularity
WCAP = 440                  # packed word columns per core (LPT max 440)
NU = WCAP // GR             # 112 units
DELTA1 = 1e-16              # nrm guard (rsqrt via exp(-0.5 ln(x+d)))
DELTA2 = 1e-12              # P2 guard
CHK = 2                     # triples per ln/exp rsqrt chunk
# region-sum groups: (first triple, n triples); rows = 3*n images <= 128
GROUPS = [(0, 42), (42, 1)]
MGX = max(n for _, n in GROUPS) * TRIP  # 126

# pipeline offsets (tile t's op runs at step t + OFF_*)
OFF_EVAC = 1   # ACT raw-G copy + DVE lrelu STT
OFF_SQ = 2     # ACT square
OFF_URED = 3   # DVE unit-reduce + DMA xbar transpose
OFF_SEGMM = 4  # PE unit->slot matmul (+ chunked ln/exp after odd member)
OFF_BCAST = 6  # PE rcp broadcast matmul + DVE an
OFF_EXP = 7    # ACT exp
OFF_UPS = 8    # PE ups matmul + GPSIMD prod1 + DVE prod2
OFF_ACC = 9    # PE P1/P2 accumulation (+ group drain)
NSTEP = NT + OFF_ACC + 1


def _group_of(t):
    for gi, (t0, ntg) in enumerate(GROUPS):
        if t0 <= t < t0 + ntg:
            return gi, t - t0
    raise ValueError(t)


def _build_nc():
    nc = bass.Bass("TRN2", target_bir_lowering=False, debug=False,
                   num_devices=NCORES)

    # register activation-bias constants (mirrors Bass.__init__'s consts)
    for v in (DELTA1, DELTA2):
        t = nc.alloc_sbuf_tensor(f"const-f32-{v}", [128, 1], F32)
        nc.gpsimd.memset(t.ap(), v)
        nc.const_aps.aps[(F32, v)] = t.ap()
    nc.all_engine_barrier()

    F8 = mybir.dt.float8e4
    imT = nc.dram_tensor("imT", [128, NT, KD * 128], F8, kind="ExternalInput")
    msbT = nc.dram_tensor("msbT", [PT, NT * PT], BF16, kind="ExternalInput")
    capT = nc.dram_tensor("capT", [128, KD, WCAP], F8, kind="ExternalInput")
    capseg_d = nc.dram_tensor("capseg", [16, WCAP], BF16, kind="ExternalInput")
    useg_d = nc.dram_tensor("unitseg", [NU, NSLOT], BF16, kind="ExternalInput")
    w1inv_d = nc.dram_tensor("w1invrow", [WCAP], BF16, kind="ExternalInput")
    mask_d = nc.dram_tensor("maskrow", [WCAP], BF16, kind="ExternalInput")
    onesb_d = nc.dram_tensor("onesb", [PT, 2 * MGX], BF16, kind="ExternalInput")
    ident_d = nc.dram_tensor("ident", [128, 128], F32, kind="ExternalInput")
    scores_d = nc.dram_tensor("scores", [NSLOT, IMG_PAD], F32, kind="ExternalOutput")

    with tile.TileContext(nc) as tc:
        with (
            tc.tile_pool(name="const", bufs=1) as const,
            tc.tile_pool(name="imt", bufs=4) as imtp,
            tc.tile_pool(name="msb", bufs=11) as msbp,
            tc.tile_pool(name="sa", bufs=7) as sap,       # a_t stream
            tc.tile_pool(name="sg", bufs=9) as sgp,       # g_sb stream
            tc.tile_pool(name="work", bufs=3) as work,    # short-lived tiles
            tc.tile_pool(name="norm", bufs=3) as normp,   # rln / rcpT chunks
            tc.tile_pool(name="drain", bufs=2) as drainp,
            tc.tile_pool(name="pg", bufs=2, space="PSUM") as pg,
            tc.tile_pool(name="pchunk", bufs=1, space="PSUM") as pchunk,
            tc.tile_pool(name="pru", bufs=2, space="PSUM") as pru,
            tc.tile_pool(name="pfill", bufs=1, space="PSUM") as pfill,
            tc.tile_pool(name="pacc", bufs=1, space="PSUM") as pacc,
        ):
            # ---- resident constants.  ident first on the sync queue (the
            # warm-up matmuls depend on it); the big imT/msbT resident loads
            # stream on the gpsimd (SWDGE) queue so the sync queue stays
            # free for the per-triple xbar transposes. ----
            ident = const.tile([128, 128], F32)
            nc.sync.dma_start(out=ident, in_=ident_d.ap())
            capseg = const.tile([16, WCAP], BF16)
            nc.sync.dma_start(out=capseg, in_=capseg_d.ap())
            useg = const.tile([NU, NSLOT], BF16)
            nc.sync.dma_start(out=useg, in_=useg_d.ap())
            w1b = const.tile([MGX, WCAP], BF16)
            nc.sync.dma_start(out=w1b, in_=w1inv_d.ap()[None, :].to_broadcast([MGX, WCAP]))
            mkb = const.tile([MGX, WCAP], BF16)
            nc.sync.dma_start(out=mkb, in_=mask_d.ap()[None, :].to_broadcast([MGX, WCAP]))
            onesb = const.tile([PT, 2 * MGX], BF16)
            nc.sync.dma_start(out=onesb, in_=onesb_d.ap())
            cap_sb = const.tile([128, KD, WCAP], F8)
            nc.gpsimd.dma_start(out=cap_sb, in_=capT.ap())

            # ---- HAM warm-up + per-step filler matmuls: the PE clock gate
            # (HAM) throttles to 4/8 (1.2 GHz) whenever the PE idles inside
            # a ~3.4us activity window.  Dependency-free filler matmuls at
            # the tail of each step's PE queue absorb any PE idle so the
            # gate stays at 8/8 (2.4 GHz). ----
            fillt = pfill.tile([128, 448], F32, name="fill")

            def fillers(n):
                # 1-col LDWEIGHTS + 128-col bf16 matmul: ~60ns each warm,
                # dependency-free (consts + dedicated PSUM bank only).
                for _ in range(n):
                    nc.tensor.matmul(fillt[:1, :128],
                                     lhsT=onesb[:PT, :1],
                                     rhs=onesb[:PT, :128], start=True,
                                     stop=True, skip_group_check=True)

            # big fp32 warm-up burst: ~4.5us of back-to-back matmuls trips
            # the HAM clock gate to 8/8 before the pipeline starts
            for _ in range(10):
                nc.tensor.matmul(fillt[:126, :128], lhsT=ident[:126, :126],
                                 rhs=ident[:126, :128], start=True, stop=True,
                                 skip_group_check=True)

            st = {}         # per-triple pipeline state
            chunks = {}     # chunk idx -> psum tile
            gacc = {}       # group -> (p1_acc, p2_acc)

            def stage_dma(t):
                """Prefetch imt (t-major, 1KB contiguous rows) + msb."""
                imt = imtp.tile([128, KD, 128], F8, tag="imt", name="imt")
                nc.sync.dma_start(
                    out=imt,
                    in_=imT.ap()[:, t, :].rearrange("p (k d) -> p k d", k=KD))
                msb = msbp.tile([PT, PT], BF16, tag="msb", name="msb")
                nc.sync.dma_start(out=msb,
                                  in_=msbT.ap()[:, t * PT:(t + 1) * PT])
                st[t] = {"imt": imt, "msb": msb}

            def emit_g(t):
                """G matmul chunks -- fp8 DoubleRow, 4 MMs of 448 cols."""
                s = st[t]
                s["gps"] = pg.tile([128, WCAP], F32, tag="G", name="gps")
                for k in range(0, KD, 2):
                    nc.tensor.matmul(s["gps"], lhsT=s["imt"][:, k:k + 2, :],
                                     rhs=cap_sb[:, k:k + 2, :],
                                     start=(k == 0), stop=(k == KD - 2),
                                     perf_mode=mybir.MatmulPerfMode.DoubleRow,
                                     skip_group_check=True)

            def stage_evac(t):
                """ACT parametric-relu (alpha=0.1, in the ln/exp table set)
                + raw-G copy, both PSUM->SBUF bf16."""
                s = st[t]
                a_t = sap.tile([PT, WCAP], BF16, tag="a")
                nc.scalar.activation(out=a_t, in_=s["gps"][:PT],
                                     func=AF.Prelu, alpha=0.1)
                g_sb = sgp.tile([PT, WCAP], BF16, tag="g")
                nc.scalar.copy(out=g_sb, in_=s["gps"][:PT])
                s["a_t"] = a_t
                s["g_sb"] = g_sb
                del s["gps"]

            def stage_sq(t):
                """DVE square: bf16 SBUF tensor_tensor runs in 2x mode."""
                s = st[t]
                sq = work.tile([PT, WCAP], BF16, tag="sq")
                nc.vector.tensor_tensor(out=sq, in0=s["a_t"], in1=s["a_t"],
                                        op=ALU.mult)
                s["sq"] = sq

            def stage_ured(t):
                """DVE 4-word unit reduce; DMA xbar transpose."""
                s = st[t]
                ured = work.tile([112, 128], BF16, tag="ured")
                with nc.allow_low_precision("bf16 unit norms; rel tol 2e-2"):
                    nc.vector.tensor_reduce(
                        out=ured[:PT, :NU],
                        in_=s.pop("sq").rearrange("p (u g) -> p u g", g=GR),
                        axis=mybir.AxisListType.X, op=ALU.add)
                utcT = work.tile([128, 112], BF16, tag="utcT")
                nc.sync.dma_start_transpose(out=utcT, in_=ured)
                s["utcT"] = utcT

            def stage_segmm(t):
                """PE unit->slot segmented matmul into the chunk bank."""
                s = st[t]
                ci, off = divmod(t, CHK)
                if off == 0:
                    chunks[ci] = pchunk.tile([16, 512], F32, tag="chunk",
                                             name="chunk")
                nc.tensor.matmul(chunks[ci][:NSLOT, off * PT:(off + 1) * PT],
                                 lhsT=useg, rhs=s["utcT"][:NU, :PT],
                                 start=True, stop=True, skip_group_check=True)
                if off == CHK - 1 or t == NT - 1:
                    n = (off + 1) * PT
                    rln = normp.tile([NSLOT, CHK * PT], F32, tag="rln")
                    nc.scalar.activation(out=rln[:, :n],
                                         in_=chunks[ci][:NSLOT, :n],
                                         func=AF.Ln, bias=DELTA1)
                    rcpT = normp.tile([NSLOT, CHK * PT], BF16, tag="rcpT")
                    nc.scalar.activation(out=rcpT[:, :n], in_=rln[:, :n],
                                         func=AF.Exp, scale=-0.5)
                    chunks.pop(ci)
                    for tt_ in range(ci * CHK, t + 1):
                        st[tt_]["rcpT"] = rcpT

            def stage_bcast(t):
                """PE rcp broadcast -> bf16 PSUM; DVE an = a * rcp (2x)."""
                s = st[t]
                off = t % CHK
                rcp = pru.tile([128, 512], F32, tag="ru", name="rcp")
                nc.tensor.matmul(rcp[:PT, :WCAP],
                                 lhsT=s.pop("rcpT")[:, off * PT:(off + 1) * PT],
                                 rhs=capseg, start=True, stop=True,
                                 skip_group_check=True)
                an = work.tile([PT, WCAP], BF16, tag="an")
                nc.vector.tensor_tensor(out=an, in0=s.pop("a_t"),
                                        in1=rcp[:PT, :WCAP], op=ALU.mult)
                s["an"] = an

            def stage_exp(t):
                s = st[t]
                e_t = work.tile([PT, WCAP], BF16, tag="E")
                nc.scalar.activation(out=e_t, in_=s.pop("an"), func=AF.Exp,
                                     scale=LAMBDA_SOFTMAX)
                s["e_t"] = e_t

            def stage_ups(t):
                """PE ups matmul; GPSIMD prod1; DVE prod2."""
                s = st[t]
                ups = pru.tile([128, 512], F32, tag="ru", name="ups")
                nc.tensor.matmul(ups[:PT, :WCAP], lhsT=s.pop("msb"),
                                 rhs=s["e_t"], start=True, stop=True,
                                 skip_group_check=True)
                prod1 = work.tile([PT, WCAP], BF16, tag="prod1")
                nc.gpsimd.tensor_tensor(out=prod1, in0=s["e_t"],
                                        in1=s.pop("g_sb"), op=ALU.mult)
                prod2 = work.tile([PT, WCAP], BF16, tag="prod2")
                nc.vector.tensor_tensor(out=prod2, in0=s.pop("e_t"),
                                        in1=ups[:PT, :WCAP], op=ALU.mult)
                s["prod1"] = prod1
                s["prod2"] = prod2

            def stage_acc(t):
                """PE P1/P2 scatter-accumulate (fp32 PSUM)."""
                s = st.pop(t)
                gi, tt = _group_of(t)
                _t0g, ntg = GROUPS[gi]
                if tt == 0:
                    gacc[gi] = (pacc.tile([MGX, WCAP], F32, tag="p1",
                                          name="p1_acc"),
                                pacc.tile([MGX, WCAP], F32, tag="p2",
                                          name="p2_acc"))
                p1_acc, p2_acc = gacc[gi]
                mg = ntg * TRIP
                lhs_ones = onesb[:, MGX - TRIP * tt:MGX - TRIP * tt + mg]
                flags = dict(start=(tt == 0), stop=(tt == ntg - 1),
                             skip_group_check=True)
                nc.tensor.matmul(p1_acc[:mg], lhsT=lhs_ones, rhs=s["prod1"],
                                 **flags)
                nc.tensor.matmul(p2_acc[:mg], lhsT=lhs_ones, rhs=s["prod2"],
                                 **flags)

            def drain(gi):
                """Per-word scores -> raw LSE sums for the group's images."""
                t0g, ntg = GROUPS[gi]
                mg = ntg * TRIP
                p1_acc, p2_acc = gacc.pop(gi)
                rsq = drainp.tile([MGX, WCAP], BF16, tag="rsq")
                rlnd = drainp.tile([MGX, WCAP], F32, tag="rlnd")
                nc.scalar.activation(out=rlnd[:mg], in_=p2_acc[:mg],
                                     func=AF.Ln, bias=DELTA2)
                nc.scalar.activation(out=rsq[:mg], in_=rlnd[:mg], func=AF.Exp,
                                     scale=-0.5)
                rs = drainp.tile([MGX, WCAP], BF16, tag="rs")
                nc.vector.tensor_tensor(out=rs[:mg], in0=p1_acc[:mg],
                                        in1=rsq[:mg], op=ALU.mult)
                nc.vector.tensor_tensor(out=rs[:mg], in0=rs[:mg],
                                        in1=w1b[:mg], op=ALU.mult)
                xx = drainp.tile([MGX, WCAP], BF16, tag="xx")
                nc.scalar.activation(out=xx[:mg], in_=rs[:mg], func=AF.Exp,
                                     scale=LAMBDA_LSE)
                nc.vector.tensor_tensor(out=xx[:mg], in0=xx[:mg],
                                        in1=mkb[:mg], op=ALU.mult)
                uredd = drainp.tile([128, 128], BF16, tag="uredd")
                with nc.allow_low_precision("bf16 LSE sums; rel tol 2e-2"):
                    nc.vector.tensor_reduce(
                        out=uredd[:mg, :NU],
                        in_=xx[:mg].rearrange("p (u g) -> p u g", g=GR),
                        axis=mybir.AxisListType.X, op=ALU.add)
                utcdT = drainp.tile([128, 128], BF16, tag="utcdT")
                nc.sync.dma_start_transpose(out=utcdT, in_=uredd)
                psd = pchunk.tile([16, 512], F32, tag="chunk", name="psd")
                nc.tensor.matmul(psd[:NSLOT, :mg], lhsT=useg,
                                 rhs=utcdT[:NU, :mg], start=True, stop=True,
                                 skip_group_check=True)
                lse_sb = drainp.tile([NSLOT, MGX], F32, tag="lse")
                nc.scalar.copy(out=lse_sb[:, :mg], in_=psd[:NSLOT, :mg])
                nc.sync.dma_start(
                    out=scores_d.ap()[:, t0g * TRIP:t0g * TRIP + mg],
                    in_=lse_sb[:, :mg])

            group_ends = {t0 + ntg - 1: gi
                          for gi, (t0, ntg) in enumerate(GROUPS)}

            # PE queue per step: ups | bcast | 4x G | P1/P2 | segmm | fillers.
            # Every PE op's cross-engine producer finished at least one full
            # step earlier; the dependency-free fillers at the tail absorb
            # any remaining PE idle so the HAM clock gate stays released.
            stage_dma(0)
            stage_dma(1)
            for s_ in range(NSTEP):
                if s_ + 2 < NT:
                    stage_dma(s_ + 2)
                if 0 <= s_ - OFF_UPS < NT:
                    stage_ups(s_ - OFF_UPS)
                fillers(1)
                if 0 <= s_ - OFF_BCAST < NT:
                    stage_bcast(s_ - OFF_BCAST)
                if s_ < NT:
                    emit_g(s_)
                if 0 <= s_ - OFF_ACC < NT:
                    stage_acc(s_ - OFF_ACC)
                if 0 <= s_ - OFF_EVAC < NT:
                    stage_evac(s_ - OFF_EVAC)
                if 0 <= s_ - OFF_SQ < NT:
                    stage_sq(s_ - OFF_SQ)
                if 0 <= s_ - OFF_URED < NT:
                    stage_ured(s_ - OFF_URED)
                fillers(1)
                if 0 <= s_ - OFF_SEGMM < NT:
                    stage_segmm(s_ - OFF_SEGMM)
                if 0 <= s_ - OFF_EXP < NT:
                    stage_exp(s_ - OFF_EXP)
                fillers(8)
                if (s_ - OFF_ACC) in group_ends:
                    drain(group_ends[s_ - OFF_ACC])

    _orig = nc.to_json_bytes
    nc.to_json_bytes = lambda *a, **k: _split_waits(_orig(*a, **k))
    return nc


_NC = None
# test-harness hooks (harmless defaults for grading)
TRACE = False
LAST_RESULTS = None


def _host_prep(im, s, s_l):
    im = np.ascontiguousarray(np.asarray(im, np.float32))
    s = np.asarray(s, np.float32)
    s_l = np.asarray(s_l).astype(np.int64)
    mask = (np.arange(W)[None, :] < s_l[:, None]).astype(np.float32)
    cap = np.ascontiguousarray(s * mask[:, :, None])
    w1 = np.sqrt(np.einsum('cwd,cwd->cw', cap, cap, dtype=np.float32,
                           optimize=True))

    imf = np.concatenate(
        [im.reshape(B * R, D), np.zeros(((IMG_PAD - B) * R, D), np.float32)], 0)
    imp = np.zeros((NT, 128, D), np.float32)
    imp[:, :PT] = imf.reshape(NT, PT, D)
    # [128 dpart, NT, KD*128]: per-triple slice is 1KB-contiguous per
    # partition -> efficient 2D DMA descriptors
    imT = np.ascontiguousarray(
        imp.reshape(NT, 128, KD, 128)       # [t, row, kd, dp]
        .transpose(3, 0, 2, 1)              # [dp, t, kd, row]
        .reshape(128, NT, KD * 128)).astype(ml_dtypes.float8_e4m3)

    # block-diagonal per-image Gram blocks, batched on host BLAS
    im4 = imf.reshape(NT, TRIP, R, D)
    gr = np.matmul(im4, im4.transpose(0, 1, 3, 2))   # [NT, TRIP, R, R]
    msb_full = np.zeros((NT, PT, PT), np.float32)
    for j in range(TRIP):
        msb_full[:, j * R:(j + 1) * R, j * R:(j + 1) * R] = gr[:, j]
    msbT = np.ascontiguousarray(
        msb_full.transpose(1, 0, 2).reshape(PT, NT * PT)).astype(
            ml_dtypes.bfloat16)

    onesb = np.zeros((PT, 2 * MGX), np.float32)
    for j in range(TRIP):
        onesb[j * R:(j + 1) * R, MGX + j] = 1.0
    onesb = onesb.astype(ml_dtypes.bfloat16)
    ident = np.eye(128, dtype=np.float32)

    # LPT assignment of captions to cores (padded-to-GR lengths)
    p4 = ((s_l + GR - 1) // GR) * GR
    order = np.argsort(-p4, kind="stable")
    loads = np.zeros(NCORES, np.int64)
    counts = np.zeros(NCORES, np.int64)
    core_caps = [[] for _ in range(NCORES)]
    for ci in order:
        elig = [c for c in range(NCORES) if counts[c] < NSLOT]
        c = min(elig, key=lambda x: loads[x])
        core_caps[c].append(int(ci))
        loads[c] += p4[ci]
        counts[c] += 1
    assert loads.max() <= WCAP, f"packing overflow: {loads.tolist()}"

    in_maps = []
    slot_map = []  # per core: list of (caption_id, n_words)
    for c in range(NCORES):
        capf = np.zeros((WCAP, D), np.float32)
        w1inv = np.zeros(WCAP, np.float32)
        mrow = np.zeros(WCAP, np.float32)
        capseg = np.zeros((NSLOT, WCAP), np.float32)
        useg = np.zeros((NU, NSLOT), np.float32)
        off = 0
        slots = []
        for j, ci in enumerate(core_caps[c]):
            l = int(s_l[ci])
            lp = int(p4[ci])
            capf[off:off + l] = cap[ci, :l]
            w1inv[off:off + l] = 1.0 / w1[ci, :l]
            mrow[off:off + l] = 1.0
            capseg[j, off:off + lp] = 1.0
            useg[off // GR:(off + lp) // GR, j] = 1.0
            slots.append((ci, l))
            off += lp
        capT = np.ascontiguousarray(
            capf.T.reshape(KD, 128, WCAP).transpose(1, 0, 2)).astype(
                ml_dtypes.float8_e4m3)
        in_maps.append({
            "imT": imT,
            "msbT": msbT,
            "capT": capT,
            "capseg": capseg.astype(ml_dtypes.bfloat16),
            "unitseg": useg.astype(ml_dtypes.bfloat16),
            "w1invrow": w1inv.astype(ml_dtypes.bfloat16),
            "maskrow": mrow.astype(ml_dtypes.bfloat16),
            "onesb": onesb,
            "ident": ident,
        })
        slot_map.append(slots)
    return in_maps, slot_map


def kernel(im, im_l, s, s_l):
    global _NC, LAST_RESULTS
    if _NC is None:
        _NC = _build_nc()
    in_maps, slot_map = _host_prep(im, s, s_l)
    res = run_bass_kernel_spmd(_NC, in_maps, core_ids=list(range(NCORES)),
                               trace=TRACE)
    LAST_RESULTS = res
    scores = np.zeros((B, B), np.float32)
    for c in range(NCORES):
        lse = res.results[c]["scores"]  # [NSLOT, IMG_PAD] raw LSE sums
        sc = np.log(np.maximum(lse[:, :B], 1e-30)) / LAMBDA_LSE
        for j, (ci, _l) in enumerate(slot_map[c]):
            scores[:, ci] = sc[j]

    diag = np.diagonal(scores)[:, None]
    cost_s = np.maximum(MARGIN + scores - diag, 0.0)
    cost_im = np.maximum(MARGIN + scores - diag.T, 0.0)
    np.fill_diagonal(cost_s, 0.0)
    np.fill_diagonal(cost_im, 0.0)
    loss = np.sum(np.max(cost_s, axis=1)) + np.sum(np.max(cost_im, axis=0))
    return np.array(loss, np.float32)
